# revision 1
# baseline (speedup 1.0000x reference)
"""Trainium2 Bass kernel for nn_CubicalModel_ISM.

Computes Xp = X @ p and Yp = Y @ p (X, Y: [784, 32768] f32, p: [32768] f32),
then gathers persistence-diagram values from the reshaped [28, 28] images.

Sharding: q (parameter) axis split across 8 NeuronCores, 4096 columns each.
Each core streams its [784, 4096] shards of X and Y through SBUF and does a
fused multiply + free-axis reduce on the Vector engine (scalar_tensor_tensor
with accum_out), producing per-core partial row sums [784] per tensor. The
[784] partials are summed across cores on the host (tiny), and the
200-element gathers run on the host as well.

Per-core layout: partition p holds rows 7p..7p+6 of the [784, 4096] shard
(112 partitions x 7 rows). Each DMA chunk moves one row per partition
([112, 4096], 16KB contiguous per partition) on the SP HWDGE ring, with the
chunk pool double/quad-buffered so DMA k+1..k+3 stream while the DVE
reduces chunk k. The measured per-core streaming rate with all 8 cores
active is ~240 GB/s, which makes the kernel DMA-bound end to end (DVE ~55%
busy); the total is within a few percent of that streaming limit.
"""

import numpy as np

H = W = 28
Q = 32768
N_CORES = 8
QS = Q // N_CORES  # 4096 per-core q shard
R = H * W          # 784 rows
P = 112            # SBUF partitions used
RPP = R // P       # 7 rows per partition

# row-chunking of the 7 rows per partition: DMA granularity.
# One row per chunk (14 DMAs of [112, 4096] = 1.79MB each, 16KB contiguous
# per partition) on a single HWDGE ring measured fastest; 2-3-row chunks,
# ring round-robin, partition-split across rings, and deeper buffering all
# measured equal-or-worse under the ~±10% run-to-run noise on this machine.
ROW_CHUNKS = [(k, k + 1) for k in range(RPP)]

_CACHE = {}


def _build_nc():
    import concourse.bacc as bacc
    import concourse.mybir as mybir
    from concourse.tile import TileContext

    # Bacc (not raw Bass) is required: its compile() runs
    # generate_event_semaphores, which splits multi-wait instructions into
    # the 1-wait-per-instruction form this walrus accepts.
    nc = bacc.Bacc(None)
    f32 = mybir.dt.float32
    x = nc.dram_tensor("x", [R, QS], f32, kind="ExternalInput")
    y = nc.dram_tensor("y", [R, QS], f32, kind="ExternalInput")
    p = nc.dram_tensor("p", [1, QS], f32, kind="ExternalInput")
    # 2*RPP full-row sums + one extra column: the very last chunk (Y row 6)
    # is processed as two half-width reduces so the final DVE op on the
    # critical tail is half as long; the host adds cols 13 and 14.
    out = nc.dram_tensor("out", [P, 2 * RPP + 1], f32, kind="ExternalOutput")

    # [784, 4096] -> [112, 7*4096]: partition p's free span = rows 7p..7p+6
    xv = x[:, :].rearrange("(p r) q -> p (r q)", p=P)
    yv = y[:, :].rearrange("(p r) q -> p (r q)", p=P)

    BANK = 512  # f32 elems per PSUM bank per partition

    with TileContext(nc) as tc:
        with (
            tc.tile_pool(name="pbpool", bufs=1) as pb_pool,
            tc.tile_pool(name="chunks", bufs=4) as chunk_pool,
            tc.tile_pool(name="scratch", bufs=1) as scratch_pool,
            tc.tile_pool(name="respool", bufs=1) as res_pool,
            tc.tile_pool(name="psum", bufs=1, space="PSUM") as psum_pool,
        ):
            p_row = pb_pool.tile([1, QS], f32)
            pb = pb_pool.tile([P, QS], f32)
            ones = pb_pool.tile([1, P], f32)
            nc.sync.dma_start(out=p_row[:, :], in_=p[:, :])
            # Broadcast p across the 112 partitions with a rank-1 matmul
            # (ones[1,112].T @ p_row[1,512] -> [112,512] per PSUM bank) and
            # ScalarE PSUM->SBUF copies. PE/ACT are otherwise idle, and this
            # avoids the GpSimd partition_broadcast custom op entirely.
            nc.vector.memset(ones[:, :], 1.0)
            pbp = psum_pool.tile([P, QS], f32)
            for k in range(QS // BANK):
                nc.tensor.matmul(
                    pbp[:, k * BANK : (k + 1) * BANK],
                    ones[:, :],
                    p_row[:, k * BANK : (k + 1) * BANK],
                    start=True,
                    stop=True,
                )
                nc.scalar.copy(
                    pb[:, k * BANK : (k + 1) * BANK],
                    pbp[:, k * BANK : (k + 1) * BANK],
                )

            res = res_pool.tile([P, 2 * RPP + 1], f32)
            scratch = scratch_pool.tile([P, QS], f32)

            def stt(in0_ap, pb_ap, col):
                # out = (in0 * 1.0) * pb elementwise (into scratch,
                # discarded); accum_out = per-partition sum — fused
                # multiply + reduce in one DVE pass.
                nc.vector.scalar_tensor_tensor(
                    out=scratch[:, : in0_ap.shape[1]],
                    in0=in0_ap,
                    scalar=1.0,
                    in1=pb_ap,
                    op0=mybir.AluOpType.mult,
                    op1=mybir.AluOpType.mult,
                    accum_out=res[:, col : col + 1],
                )

            HQ = QS // 2
            for t, src in enumerate((xv, yv)):
                for r0, r1 in ROW_CHUNKS:
                    nrows = r1 - r0
                    last = t == 1 and r1 == RPP
                    chunk = chunk_pool.tile([P, nrows * QS], f32, tag="chunk")
                    if last:
                        # Final chunk: two half-column DMAs so the first
                        # half's reduce overlaps the second half's stream,
                        # shortening the serial tail after the last byte.
                        lo = (nrows - 1) * QS
                        nc.sync.dma_start(
                            out=chunk[:, lo : lo + HQ],
                            in_=src[:, r0 * QS : r0 * QS + HQ],
                        )
                        nc.sync.dma_start(
                            out=chunk[:, lo + HQ : lo + QS],
                            in_=src[:, r0 * QS + HQ : r1 * QS],
                        )
                    else:
                        nc.sync.dma_start(
                            out=chunk[:, :], in_=src[:, r0 * QS : r1 * QS]
                        )
                    for j in range(nrows):
                        col = t * RPP + r0 + j
                        lo = j * QS
                        if last and j == nrows - 1:
                            stt(chunk[:, lo : lo + HQ], pb[:, :HQ], col)
                            stt(chunk[:, lo + HQ : lo + QS], pb[:, HQ:], 2 * RPP)
                        else:
                            stt(chunk[:, lo : lo + QS], pb[:, :], col)
            nc.sync.dma_start(out=out[:, :], in_=res[:, :])
    nc.finalize()
    return nc


def _get_nc():
    if "nc" not in _CACHE:
        _CACHE["nc"] = _build_nc()
    return _CACHE["nc"]


def _make_in_maps(X, Y, p):
    in_maps = []
    for c in range(N_CORES):
        sl = slice(c * QS, (c + 1) * QS)
        in_maps.append(
            {
                "x": np.ascontiguousarray(X[:, sl]),
                "y": np.ascontiguousarray(Y[:, sl]),
                "p": np.ascontiguousarray(p[sl]).reshape(1, QS),
            }
        )
    return in_maps


def kernel(X, Y, p, inds1, inds2):
    from concourse.bass_utils import run_bass_kernel_spmd

    X = np.asarray(X, dtype=np.float32)
    Y = np.asarray(Y, dtype=np.float32)
    p = np.asarray(p, dtype=np.float32)
    inds1 = np.asarray(inds1)
    inds2 = np.asarray(inds2)

    nc = _get_nc()
    results = run_bass_kernel_spmd(
        nc, _make_in_maps(X, Y, p), list(range(N_CORES))
    ).results

    xp = np.zeros(R, dtype=np.float32)
    yp = np.zeros(R, dtype=np.float32)
    for c in range(N_CORES):
        o = results[c]["out"]  # [112, 15]; [p, k] = row 7p + (k mod 7)
        xp += o[:, :RPP].reshape(R)
        ym = o[:, RPP : 2 * RPP].copy()
        ym[:, RPP - 1] += o[:, 2 * RPP]  # second half of Y row 7p+6
        yp += ym.reshape(R)

    def gather(img, inds):
        ij = inds.reshape(-1, 2)
        return img[ij[:, 0], ij[:, 1]].reshape(-1, 2)

    dgm1 = gather(xp.reshape(H, W), inds1)
    dgm2 = gather(yp.reshape(H, W), inds2)
    return dgm1, dgm2



# revision 2
# speedup vs baseline: 3.7214x; 3.7214x over previous
"""Trainium2 Bass kernel for nn_CubicalModel_ISM.

Reference computes Xp = X @ p and Yp = Y @ p (X, Y: [784, 32768] f32,
p: [32768] f32) and then gathers only 100 (i, j) positions from each of the
reshaped [28, 28] images.  Only the gathered rows of X and Y ever matter:
inds1/inds2 hold 100 flat row indices each, so at most 100 unique rows of X
and 100 unique rows of Y (of 784) are needed.  The host computes the unique
row sets from the (integer, data-independent) index tensors, gathers those
rows, and the device only streams ~25 MB instead of ~205 MB.

Sharding: q (parameter) axis split across 8 NeuronCores, 4096 columns each.
Per core the host packs one DRAM tensor sel[100, 8192] whose partition line
is [X_row_r | Y_row_r] for its q-shard (32 KB contiguous per partition).
The device broadcasts its p shard across the 100 partitions with a rank-1
ones-matmul (PE + ScalarE PSUM->SBUF copies), then streams sel in
[100, 1024] chunks through a quad-buffered pool, reducing each chunk with a
fused multiply + free-axis reduce (scalar_tensor_tensor accum_out) on the
DVE into one accumulator column per chunk.  Per-core partial sums
out[100, 8] are summed over cores and chunk columns on the host, and the
tiny 100-element gathers (unique-inverse mapping) finish on the host.
"""

import numpy as np

H = W = 28
Q = 32768
N_CORES = 8
QS = Q // N_CORES  # 4096 per-core q shard
NR = 100           # row capacity per tensor (= max unique gather rows)
CW = 1024          # DMA/reduce chunk width (4 KB lines)
NCH = 2 * QS // CW  # 8 chunks: 4 X + 4 Y

_CACHE = {}


def _build_nc():
    import concourse.bacc as bacc
    import concourse.mybir as mybir
    from concourse.tile import TileContext

    nc = bacc.Bacc(None)
    f32 = mybir.dt.float32
    sel = nc.dram_tensor("sel", [NR, 2 * QS], f32, kind="ExternalInput")
    p = nc.dram_tensor("p", [1, QS], f32, kind="ExternalInput")
    out = nc.dram_tensor("out", [NR, NCH], f32, kind="ExternalOutput")

    BANK = 512  # f32 elems per PSUM bank per partition

    with TileContext(nc) as tc:
        with (
            tc.tile_pool(name="pbpool", bufs=1) as pb_pool,
            tc.tile_pool(name="chunks", bufs=4) as chunk_pool,
            tc.tile_pool(name="scratch", bufs=1) as scratch_pool,
            tc.tile_pool(name="respool", bufs=1) as res_pool,
            tc.tile_pool(name="psum", bufs=1, space="PSUM") as psum_pool,
        ):
            p_row = pb_pool.tile([1, QS], f32)
            pb = pb_pool.tile([NR, QS], f32)
            ones = pb_pool.tile([1, NR], f32)
            nc.sync.dma_start(out=p_row[:, :], in_=p[:, :])
            # Broadcast p across the 100 partitions with a rank-1 matmul
            # (ones[1,100].T @ p_row[1,512] -> [100,512] per PSUM bank) and
            # ScalarE PSUM->SBUF copies; PE/ACT are otherwise idle.
            nc.vector.memset(ones[:, :], 1.0)
            pbp = psum_pool.tile([NR, QS], f32)
            for k in range(QS // BANK):
                nc.tensor.matmul(
                    pbp[:, k * BANK : (k + 1) * BANK],
                    ones[:, :],
                    p_row[:, k * BANK : (k + 1) * BANK],
                    start=True,
                    stop=True,
                )
                nc.scalar.copy(
                    pb[:, k * BANK : (k + 1) * BANK],
                    pbp[:, k * BANK : (k + 1) * BANK],
                )

            res = res_pool.tile([NR, NCH], f32)
            scratch = scratch_pool.tile([NR, CW], f32)
            for k in range(NCH):
                chunk = chunk_pool.tile([NR, CW], f32, tag="chunk")
                nc.sync.dma_start(out=chunk[:, :], in_=sel[:, k * CW : (k + 1) * CW])
                pb_off = (k * CW) % QS
                # out = (chunk * 1.0) * pb elementwise (into scratch,
                # discarded); accum_out = per-partition sum.
                nc.vector.scalar_tensor_tensor(
                    out=scratch[:, :],
                    in0=chunk[:, :],
                    scalar=1.0,
                    in1=pb[:, pb_off : pb_off + CW],
                    op0=mybir.AluOpType.mult,
                    op1=mybir.AluOpType.mult,
                    accum_out=res[:, k : k + 1],
                )
            nc.sync.dma_start(out=out[:, :], in_=res[:, :])
    nc.finalize()
    return nc


def _get_nc():
    if "nc" not in _CACHE:
        _CACHE["nc"] = _build_nc()
    return _CACHE["nc"]


def _unique_rows(inds):
    # inds: [200] int, pairs (i, j); flat row index = i*28 + j into the
    # row-major [784]-row matvec output.
    ij = np.asarray(inds).reshape(-1, 2).astype(np.int64)
    flat = ij[:, 0] * W + ij[:, 1]  # [100]
    uniq, inv = np.unique(flat, return_inverse=True)
    rows = np.full(NR, uniq[0], dtype=np.int64)
    rows[: len(uniq)] = uniq
    return rows, inv


def _make_in_maps(X, Y, p, rows1, rows2):
    Xs = X[rows1]  # [NR, Q]
    Ys = Y[rows2]
    in_maps = []
    for c in range(N_CORES):
        sl = slice(c * QS, (c + 1) * QS)
        buf = np.empty((NR, 2 * QS), dtype=np.float32)
        buf[:, :QS] = Xs[:, sl]
        buf[:, QS:] = Ys[:, sl]
        in_maps.append(
            {
                "sel": buf,
                "p": np.ascontiguousarray(p[sl]).reshape(1, QS),
            }
        )
    return in_maps


def kernel(X, Y, p, inds1, inds2):
    from concourse.bass_utils import run_bass_kernel_spmd

    X = np.asarray(X, dtype=np.float32)
    Y = np.asarray(Y, dtype=np.float32)
    p = np.asarray(p, dtype=np.float32)

    rows1, inv1 = _unique_rows(inds1)
    rows2, inv2 = _unique_rows(inds2)

    nc = _get_nc()
    results = run_bass_kernel_spmd(
        nc, _make_in_maps(X, Y, p, rows1, rows2), list(range(N_CORES))
    ).results

    acc = np.zeros((NR, NCH), dtype=np.float32)
    for c in range(N_CORES):
        acc += results[c]["out"]
    half = NCH // 2
    xsel = acc[:, :half].sum(axis=1)  # [NR] dot(X[rows1[r]], p)
    ysel = acc[:, half:].sum(axis=1)

    dgm1 = xsel[inv1].reshape(-1, 2).astype(np.float32, copy=False)
    dgm2 = ysel[inv2].reshape(-1, 2).astype(np.float32, copy=False)
    return dgm1, dgm2


# revision 4
# speedup vs baseline: 4.3132x; 1.1590x over previous
"""Trainium2 Bass kernel for nn_CubicalModel_ISM.

Reference computes Xp = X @ p and Yp = Y @ p (X, Y: [784, 32768] f32,
p: [32768] f32) and then gathers only 100 (i, j) positions from each of the
reshaped [28, 28] images.  Only the gathered rows of X and Y ever matter:
inds1/inds2 hold 100 flat row indices each, so at most 100 unique rows of X
and 100 unique rows of Y (of 784) are needed.  The host computes the unique
row sets from the (integer, data-independent) index tensors, gathers those
rows, and the device only streams ~25 MB instead of ~205 MB.

Sharding: q (parameter) axis split across 8 NeuronCores, 4096 columns each.
Per core the host packs one DRAM tensor sel[100, 8192] whose partition line
is [X_row_r | Y_row_r] for its q-shard (32 KB contiguous per partition).
The device broadcasts its p shard across the 100 partitions with a rank-1
ones-matmul (PE + ScalarE PSUM->SBUF copies), then streams sel in
[100, 1024] chunks through a quad-buffered pool, reducing each chunk with a
fused multiply + free-axis reduce (scalar_tensor_tensor accum_out) on the
DVE into one accumulator column per chunk.  Per-core partial sums
out[100, 8] are summed over cores and chunk columns on the host, and the
tiny 100-element gathers (unique-inverse mapping) finish on the host.
"""

import numpy as np

H = W = 28
Q = 32768
N_CORES = 8
QS = Q // N_CORES  # 4096 per-core q shard
NR = 100           # row capacity per tensor (= max unique gather rows)
CW = 1024          # DMA/reduce chunk width (4 KB lines)
NCH = 2 * QS // CW  # 8 chunks: 4 X + 4 Y

_CACHE = {}


def _build_nc():
    import concourse.bacc as bacc
    import concourse.mybir as mybir
    from concourse.tile import TileContext

    nc = bacc.Bacc(None)
    f32 = mybir.dt.float32
    bf16 = mybir.dt.bfloat16
    sel = nc.dram_tensor("sel", [NR, 2 * QS], f32, kind="ExternalInput")
    # p shard split into bf16 hi/lo parts on the host: ph[0] = bf16(p),
    # ph[1] = bf16(p - f32(ph[0])).  hi + lo reconstructs p to ~2^-18.
    ph = nc.dram_tensor("ph", [2, QS], bf16, kind="ExternalInput")
    out = nc.dram_tensor("out", [NR, NCH], f32, kind="ExternalOutput")

    BANK = 512  # f32 elems per PSUM bank per partition

    with TileContext(nc) as tc:
        with (
            tc.tile_pool(name="pbpool", bufs=1) as pb_pool,
            tc.tile_pool(name="chunks", bufs=6) as chunk_pool,
            tc.tile_pool(name="scratch", bufs=1) as scratch_pool,
            tc.tile_pool(name="respool", bufs=1) as res_pool,
            tc.tile_pool(name="psum", bufs=1, space="PSUM") as psum_pool,
        ):
            p_row = pb_pool.tile([2, QS], bf16)
            pb = pb_pool.tile([NR, QS], f32)
            ones = pb_pool.tile([2, NR], bf16)
            nc.sync.dma_start(out=p_row[:, :], in_=ph[:, :])
            # Broadcast p across the 100 partitions with a K=2 rank-2 bf16
            # matmul: ones[2,100].T @ [p_hi; p_lo][2,512] -> [100,512] per
            # PSUM bank (f32 accumulation adds hi+lo back together), then
            # ScalarE PSUM->SBUF copies.  bf16 streams at 1 cycle/row on the
            # PE vs 4 for fp32, so the broadcast no longer starves the DVE.
            nc.vector.memset(ones[:, :], 1.0)
            pbp = psum_pool.tile([NR, QS], f32)
            for k in range(QS // BANK):
                nc.tensor.matmul(
                    pbp[:, k * BANK : (k + 1) * BANK],
                    ones[:, :],
                    p_row[:, k * BANK : (k + 1) * BANK],
                    start=True,
                    stop=True,
                )
                nc.scalar.copy(
                    pb[:, k * BANK : (k + 1) * BANK],
                    pbp[:, k * BANK : (k + 1) * BANK],
                )

            res = res_pool.tile([NR, NCH], f32)
            scratch = scratch_pool.tile([NR, CW], f32)
            for k in range(NCH):
                chunk = chunk_pool.tile([NR, CW], f32, tag="chunk")
                nc.sync.dma_start(out=chunk[:, :], in_=sel[:, k * CW : (k + 1) * CW])
                pb_off = (k * CW) % QS
                # out = (chunk * 1.0) * pb elementwise (into scratch,
                # discarded); accum_out = per-partition sum.
                nc.vector.scalar_tensor_tensor(
                    out=scratch[:, :],
                    in0=chunk[:, :],
                    scalar=1.0,
                    in1=pb[:, pb_off : pb_off + CW],
                    op0=mybir.AluOpType.mult,
                    op1=mybir.AluOpType.mult,
                    accum_out=res[:, k : k + 1],
                )
            nc.sync.dma_start(out=out[:, :], in_=res[:, :])
    nc.finalize()
    return nc


def _get_nc():
    if "nc" not in _CACHE:
        _CACHE["nc"] = _build_nc()
    return _CACHE["nc"]


def _unique_rows(inds):
    # inds: [200] int, pairs (i, j); flat row index = i*28 + j into the
    # row-major [784]-row matvec output.
    ij = np.asarray(inds).reshape(-1, 2).astype(np.int64)
    flat = ij[:, 0] * W + ij[:, 1]  # [100]
    uniq, inv = np.unique(flat, return_inverse=True)
    rows = np.full(NR, uniq[0], dtype=np.int64)
    rows[: len(uniq)] = uniq
    return rows, inv


def _make_in_maps(X, Y, p, rows1, rows2):
    import ml_dtypes

    bf16 = ml_dtypes.bfloat16
    p_hi = p.astype(bf16)
    p_lo = (p - p_hi.astype(np.float32)).astype(bf16)
    Xs = X[rows1]  # [NR, Q]
    Ys = Y[rows2]
    in_maps = []
    for c in range(N_CORES):
        sl = slice(c * QS, (c + 1) * QS)
        buf = np.empty((NR, 2 * QS), dtype=np.float32)
        buf[:, :QS] = Xs[:, sl]
        buf[:, QS:] = Ys[:, sl]
        ph = np.empty((2, QS), dtype=bf16)
        ph[0] = p_hi[sl]
        ph[1] = p_lo[sl]
        in_maps.append({"sel": buf, "ph": ph})
    return in_maps


def kernel(X, Y, p, inds1, inds2):
    from concourse.bass_utils import run_bass_kernel_spmd

    X = np.asarray(X, dtype=np.float32)
    Y = np.asarray(Y, dtype=np.float32)
    p = np.asarray(p, dtype=np.float32)

    rows1, inv1 = _unique_rows(inds1)
    rows2, inv2 = _unique_rows(inds2)

    nc = _get_nc()
    results = run_bass_kernel_spmd(
        nc, _make_in_maps(X, Y, p, rows1, rows2), list(range(N_CORES))
    ).results

    acc = np.zeros((NR, NCH), dtype=np.float32)
    for c in range(N_CORES):
        acc += results[c]["out"]
    half = NCH // 2
    xsel = acc[:, :half].sum(axis=1)  # [NR] dot(X[rows1[r]], p)
    ysel = acc[:, half:].sum(axis=1)

    dgm1 = xsel[inv1].reshape(-1, 2).astype(np.float32, copy=False)
    dgm2 = ysel[inv2].reshape(-1, 2).astype(np.float32, copy=False)
    return dgm1, dgm2
